# revision 3
# baseline (speedup 1.0000x reference)
"""Causal attention head (B=8, C=2048, E=1024, H=64) with post-softmax query-row
zero mask, on 8 TRN2 NeuronCores — data-parallel over batch (one batch per core).

Sparse trick: ~50% of query rows are zero-masked post-softmax, so their outputs
are never needed. The host gathers the kept query positions (sorted), pads them
at the FRONT to a fixed QK_PAD, and the device computes attention only for
gathered query columns. Causality for gathered columns is enforced by
(a) restricting each score tile's moving range to columns whose position can
reach that key chunk (host-baked, min over cores) and (b) one narrow host-built
0/1 mask multiply per boundary tile (per-core data).

Pipeline layout (v2): DMA triggers are ordered so consumers unblock in
program order (xt0 halves, xq[wq|q0|masks|q1], xq[q2], xt1, xt2, xt3 halves);
attention j-segments are emitted between kv-projection chunks as their k/q
inputs land, so the PE queue never blocks on a far-future dependency and the
Scalar engine's exp stream (the attention-phase bottleneck, ~(N+352)/1.2 ns
per instruction) starts as early as possible and never drains. k is projected
once and duplicated into both SBUF partition halves by GpSimd (off the
PE/Vector critical path); q is projected into both halves at once by a
column-tiled self-pair matmul, so one [128,w] cast replaces two copies.

Per-core dataflow (matmuls bf16/fp8 -> f32 PSUM):
  [Wk|Wv] packed projection over all 2048 key positions -> k_sb, vt_sb.
  v^T PE-transposed into v_aug tiles [128j, 65] with column 64 = 1.0 (the AV
        matmul then also emits softmax denominators as row 64).
  Wq projection over gathered x columns (fp8; quantization error attenuated
        ~45x by the C**-0.5 score scale) -> q_sb.
  scoresT[j, q] = k-chunk (stationary, row-tiled across PE array halves) @ q
        (moving); exp on ScalarE with the C**-0.5 scale fused, two j-tiles per
        exp op when the garbage span is under 352 columns; AV accumulates
        outT[65, q] over j-chunks (row 64 = softmax denominators).
  The host divides by the denominators and scatters columns back to rows
  (masked rows stay zero) while unsharding. Output ships bf16 (unnormalized
  sums; the host divide restores precision headroom).
"""

import numpy as np
import ml_dtypes

import concourse.bass as bass
import concourse.bacc as bacc
import concourse.mybir as mybir
import concourse.tile as tile
from concourse.bass_utils import run_bass_kernel_spmd
from concourse.masks import make_identity

B, C, E, H = 8, 2048, 1024, 64
EC = E // 128          # 8 contraction chunks
KC = C // 512          # 4 key/value column chunks of 512
NJ = C // 128          # 16 key chunks of 128
QK_PAD = 1536          # gathered queries padded (front) to this
QKC = QK_PAD // 512    # 3 gathered-query chunks
SCALE = float(C) ** -0.5
BF16 = mybir.dt.bfloat16
F32 = mybir.dt.float32

_CACHED = {}


def _plan(zero_mask):
    """Host-side plan: per-core gathered positions + shared baked bounds."""
    zm = np.asarray(zero_mask)
    pos = []   # per core: [QK_PAD] int, -1 for front pads
    for b in range(B):
        kept = np.nonzero(~zm[b])[0]
        assert len(kept) <= QK_PAD, len(kept)
        p = np.full(QK_PAD, -1, dtype=np.int64)
        p[QK_PAD - len(kept):] = kept
        pos.append(p)
    pos = np.stack(pos)  # [B, QK_PAD]

    # qoff[b, ck, jc] = #cols in chunk ck with pos < 128*jc (cols are sorted)
    qoff = np.zeros((B, QKC, NJ + 1), dtype=np.int64)
    for ck in range(QKC):
        pc = pos[:, ck * 512:(ck + 1) * 512]
        for jc in range(NJ + 1):
            qoff[:, ck, jc] = (pc < 128 * jc).sum(axis=1)
    jmax = []   # per chunk: number of key chunks any core needs
    mm_off = []  # baked matmul start col (min over cores)
    mk_end = []  # baked mask end col (max over cores)
    for ck in range(QKC):
        jm = 0
        for jc in range(NJ):
            if (qoff[:, ck, jc] < 512).any():
                jm = jc + 1
        jmax.append(jm)
        mm_off.append([int(qoff[:, ck, jc].min()) for jc in range(NJ)])
        mk_end.append([int(qoff[:, ck, jc + 1].max()) for jc in range(NJ)])
    return pos, qoff, tuple(jmax), mm_off, mk_end


def _build(jmax, mm_off, mk_end, mask_w):
    nc = bacc.Bacc("TRN2", target_bir_lowering=False, debug=False, num_devices=B)
    FP8 = mybir.dt.float8e4
    MW = max(mask_w, 1)
    W0F = mm_off[0][0] & ~3       # pad columns trimmed from xq chunk 0
    QW = [512 - W0F, 512, 512]    # shipped query-chunk widths
    XOFF = EC * 128   # wkv columns at the head of xt
    QOFF = EC * H     # wq columns at the head of xq
    # xq layout: [wq | xq0 (trimmed) | masks | xq1 | xq2]
    QB = [QOFF]                     # per-chunk column base inside xq
    QB.append(QOFF + EC * QW[0] + MW)
    QB.append(QB[1] + EC * 512)
    MSK0 = QOFF + EC * QW[0]        # masks sit between xq0 and xq1
    XQW = QB[2] + EC * 512
    xt_ext = nc.dram_tensor("xt", [128, XOFF + KC * EC * 512], BF16, kind="ExternalInput")
    xq_ext = nc.dram_tensor("xq", [128, XQW], FP8, kind="ExternalInput")
    out_ext = nc.dram_tensor("out", [H + 1, QK_PAD], BF16, kind="ExternalOutput")

    with tile.TileContext(nc) as tc:
        with (
            tc.tile_pool(name="const", bufs=1) as const_pool,
            tc.tile_pool(name="acts", bufs=1) as act_pool,
            tc.tile_pool(name="p", bufs=4) as p_pool,
            tc.tile_pool(name="osb", bufs=2) as o_pool,
            tc.tile_pool(name="mmp", bufs=2, space="PSUM") as mmp_pool,
            tc.tile_pool(name="mms", bufs=2, space="PSUM") as mms_pool,
            tc.tile_pool(name="po", bufs=2, space="PSUM") as po_pool,
        ):
            ident = const_pool.tile([128, 128], BF16)
            xt_all = act_pool.tile([128, XOFF + KC * EC * 512], BF16)
            xq_all = act_pool.tile([128, XQW], FP8)
            msk_sb = xq_all[:, MSK0:MSK0 + MW]
            wkv_sb = xt_all[:, 0:XOFF]
            xt_sb = xt_all[:, XOFF:]
            wq_sb = xq_all[:, 0:QOFF]

            # ---- input DMA triggers, in consumer order (sync queue) ----
            half = EC * 512 // 2

            def xt_rng(lo, hi):
                nc.sync.dma_start(xt_all[:, lo:hi], xt_ext.ap()[:, lo:hi])

            xt_rng(0, XOFF + half)                        # wkv + xt0 e0-3
            xt_rng(XOFF + half, XOFF + EC * 512)          # xt0 e4-7
            nc.sync.dma_start(                            # wq + xq0 + masks + xq1
                xq_all[:, 0:QB[2]], xq_ext.ap()[:, 0:QB[2]])
            nc.sync.dma_start(                            # xq2
                xq_all[:, QB[2]:XQW], xq_ext.ap()[:, QB[2]:XQW])
            xt_rng(XOFF + EC * 512, XOFF + 2 * EC * 512)  # xt1
            xt_rng(XOFF + 2 * EC * 512, XOFF + 3 * EC * 512)  # xt2
            xt_rng(XOFF + 3 * EC * 512, XOFF + 3 * EC * 512 + half)  # xt3 e0-3
            xt_rng(XOFF + 3 * EC * 512 + half, XOFF + 4 * EC * 512)  # xt3 e4-7

            make_identity(nc, ident[:])
            # touch Exp once so the ACT table set loads during the DMA phase
            warm = const_pool.tile([1, 1], F32)
            nc.scalar.activation(warm[:], ident[0:1, 0:1],
                                 mybir.ActivationFunctionType.Exp)

            # k and q live duplicated in both partition halves so score matmuls
            # (K=64) can run row-tiled: even j-chunks use array rows 0:64, odd
            # j-chunks rows 64:128 — the two halves compute concurrently.
            k_sb = act_pool.tile([128, C], BF16)
            vt_sb = act_pool.tile([64, C], BF16)
            q_sb = act_pool.tile([128, QK_PAD], BF16)
            vaug_sb = act_pool.tile([128, NJ * (H + 1)], BF16)
            nc.vector.memset(vaug_sb[:], 1.0)

            I32 = mybir.dt.int32

            def kv_mm(c, halves=1):
                csl = slice(c * 512, (c + 1) * 512)
                pq = mmp_pool.tile([128, 512], F32, tag="mm")
                for e in range(EC):
                    nc.tensor.matmul(
                        pq[:], wkv_sb[:, e * 128:(e + 1) * 128],
                        xt_sb[:, (c * EC + e) * 512:(c * EC + e + 1) * 512],
                        start=(e == 0), stop=(e == EC - 1))
                return pq, csl

            def kv_cast(c, pq, csl):
                nc.vector.tensor_copy(k_sb[0:64, csl], pq[0:64, :])
                nc.vector.tensor_copy(vt_sb[:, csl], pq[64:128, :])
                # duplicate k into the upper partition half off the critical
                # engines (SBUF->SBUF, as int32 for the packed-copy fast path)
                nc.gpsimd.tensor_copy(
                    k_sb[64:128, csl].bitcast(I32), k_sb[0:64, csl].bitcast(I32))

            def trp(c):
                for jj in range(4):
                    jc = 4 * c + jj
                    pt = mmp_pool.tile([128, H], BF16, tag="mm")
                    nc.tensor.transpose(
                        pt[:], vt_sb[:, jc * 128:(jc + 1) * 128],
                        ident[0:64, 0:64])
                    nc.vector.tensor_copy(
                        vaug_sb[:, jc * (H + 1): jc * (H + 1) + H], pt[:])

            def q_proj(ck):
                # self-paired: the same chunk accumulates into both PSUM
                # partition halves concurrently (column-tiled), so a single
                # [128, w] cast lands q duplicated in both SBUF halves.
                w = QW[ck]
                base = QB[ck]
                pv = mmp_pool.tile([128, 512], F32, tag="mm")
                for e in range(EC):
                    xsl = xq_all[:, base + e * w:base + (e + 1) * w]
                    nc.tensor.matmul(
                        pv[0:64, 0:w], wq_sb[:, e * H:(e + 1) * H], xsl,
                        start=(e == 0), stop=(e == EC - 1),
                        skip_group_check=True)
                    nc.tensor.matmul(
                        pv[64:128, 0:w], wq_sb[:, e * H:(e + 1) * H], xsl,
                        start=(e == 0), stop=(e == EC - 1),
                        skip_group_check=True)
                lo = ck * 512 + (512 - w)
                nc.vector.tensor_copy(q_sb[:, lo:(ck + 1) * 512], pv[:, 0:w])

            # ---- attention machinery ----
            # mask tile packing offsets (shared layout; content is per-core)
            mask_offs = {}
            off = 0
            for ck in range(QKC):
                for jc in range(jmax[ck]):
                    qo, me = mm_off[ck][jc], mk_end[ck][jc]
                    if me > qo and qo < 512:
                        mask_offs[(ck, jc)] = off
                        off += me - qo

            tiles = {ck: [(jc, mm_off[ck][jc], mk_end[ck][jc])
                          for jc in range(jmax[ck]) if mm_off[ck][jc] < 512]
                     for ck in range(QKC)}
            po_state = {}  # ck -> [po_tile, n_av_done, n_av_total]

            def att_seg(ck, j_lo, j_hi):
                seg = [t for t in tiles[ck] if j_lo <= t[0] <= j_hi]
                if not seg:
                    return
                if ck not in po_state:
                    po_state[ck] = [po_pool.tile([H + 1, 512], F32, tag="po",
                                                 name=f"po{ck}"),
                                    0, len(tiles[ck])]
                st = po_state[ck]
                i = 0
                while i < len(seg):
                    pair = seg[i:i + 2]
                    ps = mms_pool.tile([128, 1024], F32, tag="mms")
                    p_t = p_pool.tile([128, 1024], BF16, tag="p")
                    for h, (jc, qo, me) in enumerate(pair):
                        hf = 64 * (jc % 2)  # row-tiled: alternate array halves
                        nc.tensor.matmul(
                            ps[:, h * 512 + qo:(h + 1) * 512],
                            k_sb[hf:hf + 64, jc * 128:(jc + 1) * 128],
                            q_sb[hf:hf + 64, ck * 512 + qo:(ck + 1) * 512],
                            start=True, stop=True, skip_group_check=True)
                    if len(pair) == 2 and pair[1][1] < 352:
                        lo = pair[0][1]
                        nc.scalar.activation(
                            p_t[:, lo:1024], ps[:, lo:1024],
                            mybir.ActivationFunctionType.Exp, scale=SCALE)
                    else:
                        for h, (jc, qo, me) in enumerate(pair):
                            nc.scalar.activation(
                                p_t[:, h * 512 + qo:(h + 1) * 512],
                                ps[:, h * 512 + qo:(h + 1) * 512],
                                mybir.ActivationFunctionType.Exp, scale=SCALE)
                    for h, (jc, qo, me) in enumerate(pair):
                        if me > qo:  # boundary mask multiply (host-built)
                            mo = mask_offs[(ck, jc)]
                            nc.vector.tensor_mul(
                                p_t[:, h * 512 + qo:h * 512 + me],
                                p_t[:, h * 512 + qo:h * 512 + me],
                                msk_sb[:, mo:mo + (me - qo)])
                        nc.tensor.matmul(
                            st[0][:, qo:512],
                            vaug_sb[:, jc * (H + 1):(jc + 1) * (H + 1)],
                            p_t[:, h * 512 + qo:(h + 1) * 512],
                            start=(st[1] == 0), stop=(st[1] == st[2] - 1))
                        st[1] += 1
                    i += len(pair)

            def att_close(ck):
                # ship unnormalized outT + sums row; the host divides while
                # unsharding (removes the recip chain from the critical tail)
                w0 = mm_off[ck][0]
                o_t = o_pool.tile([H + 1, 512], BF16, tag="o")
                nc.vector.tensor_copy(o_t[:, w0:512], po_state[ck][0][:, w0:512])
                nc.sync.dma_start(
                    out_ext.ap()[:, ck * 512 + w0:(ck + 1) * 512], o_t[:, w0:512])

            # ---- schedule: consumers emitted as their inputs land ----
            pq0, csl0 = kv_mm(0)
            kv_cast(0, pq0, csl0)
            q_proj(0)
            q_proj(1)
            trp(0)
            att_seg(0, 0, 0)
            att_close(0)
            att_seg(1, 0, 3)
            q_proj(2)
            att_seg(2, 0, 3)
            pq1, csl1 = kv_mm(1)
            kv_cast(1, pq1, csl1)
            trp(1)
            att_seg(1, 4, 7)
            att_seg(2, 4, 7)
            pq2, csl2 = kv_mm(2)
            kv_cast(2, pq2, csl2)
            trp(2)
            att_seg(1, 8, 8)
            att_close(1)
            att_seg(2, 8, 11)
            pq3, csl3 = kv_mm(3)
            kv_cast(3, pq3, csl3)
            trp(3)
            att_seg(2, 12, 15)
            att_close(2)

    nc.compile()
    return nc


def _pack_masks(pos, jmax, mm_off, mk_end):
    """Per-core packed boundary masks: msk[j_local, off+q-qo] = (128jc + j_local <= pos[q])."""
    total = 0
    spans = []
    for ck in range(len(jmax)):
        for jc in range(jmax[ck]):
            qo, me = mm_off[ck][jc], mk_end[ck][jc]
            if me > qo and qo < 512:
                spans.append((ck, jc, qo, me, total))
                total += me - qo
    bf = ml_dtypes.bfloat16
    masks = np.zeros((B, 128, max(total, 1)), dtype=np.float32)
    jl = np.arange(128)[:, None]
    for b in range(B):
        for ck, jc, qo, me, off in spans:
            pq = pos[b, ck * 512 + qo: ck * 512 + me][None, :]
            masks[b, :, off:off + (me - qo)] = (128 * jc + jl <= pq)
    return masks.astype(bf), total


def _sbufify(w):  # [E, M] -> [128, EC*M]: w_t[p, e*M+m] = w[e*128+p, m]
    M = w.shape[1]
    return np.ascontiguousarray(
        w.reshape(EC, 128, M).transpose(1, 0, 2).reshape(128, EC * M))


def _retile_cols(xt, ncols, w=512):  # [E, ncols] -> [128, (ncols/w)*EC*w] chunk-major
    return np.ascontiguousarray(
        xt.reshape(EC, 128, ncols // w, w).transpose(1, 2, 0, 3)
        .reshape(128, (ncols // w) * EC * w))


def make_in_maps(x, Wq, Wk, Wv, zero_mask):
    x = np.asarray(x)
    pos, qoff, jmax, mm_off, mk_end = _plan(zero_mask)
    masks, mask_w = _pack_masks(pos, jmax, mm_off, mk_end)
    bf = ml_dtypes.bfloat16
    f8 = ml_dtypes.float8_e4m3fn
    W0F = mm_off[0][0] & ~3
    wkv = _sbufify(np.concatenate([np.asarray(Wk), np.asarray(Wv)], 1)).astype(bf)
    wq = _sbufify(np.asarray(Wq)).astype(f8)
    maps = []
    for b in range(B):
        xtb = np.ascontiguousarray(x[b].T.astype(np.float32))
        xqb = np.zeros((E, QK_PAD), dtype=np.float32)
        real = pos[b] >= 0
        xqb[:, real] = xtb[:, pos[b][real]]
        # chunk 0 ships trimmed (cols W0F:512 only; the rest are pads on
        # every core), chunks 1-2 full width
        xq0 = np.ascontiguousarray(
            xqb[:, W0F:512].reshape(EC, 128, 512 - W0F)
            .transpose(1, 0, 2).reshape(128, EC * (512 - W0F))).astype(f8)
        xq12 = _retile_cols(xqb[:, 512:], 1024).astype(f8)
        xq_packed = np.concatenate(  # [wq | xq0 | masks | xq1 | xq2]
            [wq, xq0, masks[b].astype(f8), xq12], axis=1)
        xt_packed = np.concatenate([wkv, _retile_cols(xtb, C).astype(bf)], axis=1)
        maps.append({
            "xt": np.ascontiguousarray(xt_packed),
            "xq": np.ascontiguousarray(xq_packed),
        })
    return maps, (pos, jmax, mm_off, mk_end, mask_w)


def kernel(x, Wq, Wk, Wv, zero_mask):
    in_maps, (pos, jmax, mm_off, mk_end, mask_w) = make_in_maps(
        x, Wq, Wk, Wv, zero_mask)
    key = (jmax, tuple(map(tuple, mm_off)), tuple(map(tuple, mk_end)), mask_w)
    if _CACHED.get("key") != key:
        _CACHED["nc"] = _build(jmax, mm_off, mk_end, mask_w)
        _CACHED["key"] = key
    res = run_bass_kernel_spmd(_CACHED["nc"], in_maps, core_ids=list(range(B)))
    out = np.zeros((B, C, H), dtype=np.float32)
    for b in range(B):
        r = res.results[b]["out"].astype(np.float32)  # [H+1, QK_PAD]
        real = pos[b] >= 0
        out[b][pos[b][real]] = (r[:H, real] / r[H:H + 1, real]).T
    return out


# revision 8
# speedup vs baseline: 1.0708x; 1.0708x over previous
"""Causal attention head (B=8, C=2048, E=1024, H=64) with post-softmax query-row
zero mask, on 8 TRN2 NeuronCores — data-parallel over batch (one batch per core).

Sparse trick: ~50% of query rows are zero-masked post-softmax, so their outputs
are never needed. The host gathers the kept query positions (sorted), pads them
at the FRONT to a fixed QK_PAD, and the device computes attention only for
gathered query columns. Causality for gathered columns is enforced by
(a) restricting each score tile's moving range to columns whose position can
reach that key chunk (host-baked, min over cores) and (b) one narrow host-built
0/1 mask multiply per boundary tile (per-core data).

Pipeline layout (v2): DMA triggers are ordered so consumers unblock in
program order (xt0 halves, xq[wq|q0|masks|q1], xq[q2], xt1, xt2, xt3 halves);
attention j-segments are emitted between kv-projection chunks as their k/q
inputs land, so the PE queue never blocks on a far-future dependency and the
Scalar engine's exp stream (the attention-phase bottleneck, ~(N+352)/1.2 ns
per instruction) starts as early as possible and never drains. k is projected
once and duplicated into both SBUF partition halves by GpSimd (off the
PE/Vector critical path); q is projected into both halves at once by a
column-tiled self-pair matmul, so one [128,w] cast replaces two copies.

Per-core dataflow (matmuls bf16/fp8 -> f32 PSUM):
  [Wk|Wv] packed projection over all 2048 key positions -> k_sb, vt_sb.
  v^T PE-transposed into v_aug tiles [128j, 65] with column 64 = 1.0 (the AV
        matmul then also emits softmax denominators as row 64).
  Wq projection over gathered x columns (fp8; quantization error attenuated
        ~45x by the C**-0.5 score scale) -> q_sb.
  scoresT[j, q] = k-chunk (stationary, row-tiled across PE array halves) @ q
        (moving); exp on ScalarE with the C**-0.5 scale fused, two j-tiles per
        exp op when the garbage span is under 352 columns; AV accumulates
        outT[65, q] over j-chunks (row 64 = softmax denominators).
  The host divides by the denominators and scatters columns back to rows
  (masked rows stay zero) while unsharding. Output ships bf16 (unnormalized
  sums; the host divide restores precision headroom).
"""

import numpy as np
import ml_dtypes

import concourse.bass as bass
import concourse.bacc as bacc
import concourse.mybir as mybir
import concourse.tile as tile
from concourse.bass_utils import run_bass_kernel_spmd
from concourse.masks import make_identity

B, C, E, H = 8, 2048, 1024, 64
EC = E // 128          # 8 contraction chunks
KC = C // 512          # 4 key/value column chunks of 512
NJ = C // 128          # 16 key chunks of 128
QK_PAD = 1536          # gathered queries padded (front) to this
QKC = QK_PAD // 512    # 3 gathered-query chunks
SCALE = float(C) ** -0.5
BF16 = mybir.dt.bfloat16
F32 = mybir.dt.float32

_CACHED = {}


def _plan(zero_mask):
    """Host-side plan: per-core gathered positions + shared baked bounds."""
    zm = np.asarray(zero_mask)
    pos = []   # per core: [QK_PAD] int, -1 for front pads
    for b in range(B):
        kept = np.nonzero(~zm[b])[0]
        assert len(kept) <= QK_PAD, len(kept)
        p = np.full(QK_PAD, -1, dtype=np.int64)
        p[QK_PAD - len(kept):] = kept
        pos.append(p)
    pos = np.stack(pos)  # [B, QK_PAD]

    # qoff[b, ck, jc] = #cols in chunk ck with pos < 128*jc (cols are sorted)
    qoff = np.zeros((B, QKC, NJ + 1), dtype=np.int64)
    for ck in range(QKC):
        pc = pos[:, ck * 512:(ck + 1) * 512]
        for jc in range(NJ + 1):
            qoff[:, ck, jc] = (pc < 128 * jc).sum(axis=1)
    jmax = []   # per chunk: number of key chunks any core needs
    mm_off = []  # baked matmul start col (min over cores)
    mk_end = []  # baked mask end col (max over cores)
    for ck in range(QKC):
        jm = 0
        for jc in range(NJ):
            if (qoff[:, ck, jc] < 512).any():
                jm = jc + 1
        jmax.append(jm)
        mm_off.append([int(qoff[:, ck, jc].min()) for jc in range(NJ)])
        mk_end.append([int(qoff[:, ck, jc + 1].max()) for jc in range(NJ)])
    return pos, qoff, tuple(jmax), mm_off, mk_end


def _build(jmax, mm_off, mk_end, mask_w):
    nc = bacc.Bacc("TRN2", target_bir_lowering=False, debug=False, num_devices=B)
    FP8 = mybir.dt.float8e4
    MW = max(mask_w, 1)
    W0F = mm_off[0][0] & ~3       # pad columns trimmed from xq chunk 0
    QW = [512 - W0F, 512, 512]    # shipped query-chunk widths
    XOFF = EC * 128   # wkv columns at the head of xt
    QOFF = EC * 128   # [wq|wq] columns at the head of xq (duplicated: one
                      # matmul then emits q into both PSUM partition halves)
    # xq layout: [wq | xq0 (trimmed) | masks | xq1 | xq2]
    QB = [QOFF]                     # per-chunk column base inside xq
    QB.append(QOFF + EC * QW[0] + MW)
    QB.append(QB[1] + EC * 512)
    MSK0 = QOFF + EC * QW[0]        # masks sit between xq0 and xq1
    XQW = QB[2] + EC * 512
    xt_ext = nc.dram_tensor("xt", [128, XOFF + KC * EC * 512], BF16, kind="ExternalInput")
    xq_ext = nc.dram_tensor("xq", [128, XQW], FP8, kind="ExternalInput")
    out_ext = nc.dram_tensor("out", [H + 1, QK_PAD], BF16, kind="ExternalOutput")

    with tile.TileContext(nc) as tc:
        with (
            tc.tile_pool(name="const", bufs=1) as const_pool,
            tc.tile_pool(name="acts", bufs=1) as act_pool,
            tc.tile_pool(name="p", bufs=4) as p_pool,
            tc.tile_pool(name="osb", bufs=2) as o_pool,
            tc.tile_pool(name="mmp", bufs=2, space="PSUM") as mmp_pool,
            tc.tile_pool(name="mms", bufs=2, space="PSUM") as mms_pool,
            tc.tile_pool(name="po", bufs=2, space="PSUM") as po_pool,
        ):
            ident = const_pool.tile([128, 128], BF16)
            xt_all = act_pool.tile([128, XOFF + KC * EC * 512], BF16)
            xq_all = act_pool.tile([128, XQW], FP8)
            msk_sb = xq_all[:, MSK0:MSK0 + MW]
            wkv_sb = xt_all[:, 0:XOFF]
            xt_sb = xt_all[:, XOFF:]
            wq_sb = xq_all[:, 0:QOFF]

            # ---- input DMA triggers, in consumer order (sync queue) ----
            half = EC * 512 // 2

            def xt_rng(lo, hi):
                nc.sync.dma_start(xt_all[:, lo:hi], xt_ext.ap()[:, lo:hi])

            xt_rng(0, XOFF + half)                        # wkv + xt0 e0-3
            xt_rng(XOFF + half, XOFF + EC * 512)          # xt0 e4-7
            nc.sync.dma_start(                            # wq + xq0 + masks + xq1
                xq_all[:, 0:QB[2]], xq_ext.ap()[:, 0:QB[2]])
            nc.sync.dma_start(                            # xq2
                xq_all[:, QB[2]:XQW], xq_ext.ap()[:, QB[2]:XQW])
            xt_rng(XOFF + EC * 512, XOFF + 2 * EC * 512)  # xt1
            xt_rng(XOFF + 2 * EC * 512, XOFF + 3 * EC * 512)  # xt2
            xt_rng(XOFF + 3 * EC * 512, XOFF + 3 * EC * 512 + half)  # xt3 e0-3
            xt_rng(XOFF + 3 * EC * 512 + half, XOFF + 4 * EC * 512)  # xt3 e4-7

            make_identity(nc, ident[:])
            # touch Exp once so the ACT table set loads during the DMA phase
            warm = const_pool.tile([1, 1], F32)
            nc.scalar.activation(warm[:], ident[0:1, 0:1],
                                 mybir.ActivationFunctionType.Exp)

            # k and q live duplicated in both partition halves so score matmuls
            # (K=64) can run row-tiled: even j-chunks use array rows 0:64, odd
            # j-chunks rows 64:128 — the two halves compute concurrently.
            k_sb = act_pool.tile([128, C], BF16)
            vt_sb = act_pool.tile([64, C], BF16)
            q_sb = act_pool.tile([128, QK_PAD], BF16)
            vaug_sb = act_pool.tile([128, NJ * (H + 1)], BF16)
            nc.vector.memset(vaug_sb[:], 1.0)

            I32 = mybir.dt.int32

            def kv_group(c):
                csl = slice(c * 512, (c + 1) * 512)
                pq = mmp_pool.tile([128, 512], F32, tag="mm", name=f"pq{c}")
                for e in range(EC):
                    nc.tensor.matmul(
                        pq[:], wkv_sb[:, e * 128:(e + 1) * 128],
                        xt_sb[:, (c * EC + e) * 512:(c * EC + e + 1) * 512],
                        start=(e == 0), stop=(e == EC - 1))
                nc.vector.tensor_copy(k_sb[0:64, csl], pq[0:64, :])
                # vt0 lands on Scalar (idle before the exp stream starts);
                # later chunks keep Vector
                if c == 0:
                    nc.scalar.copy(vt_sb[:, csl], pq[64:128, :])
                else:
                    nc.vector.tensor_copy(vt_sb[:, csl], pq[64:128, :])
                # duplicate k into the upper partition half off the critical
                # engines (SBUF->SBUF, as int32 for the packed-copy fast path)
                nc.gpsimd.tensor_copy(
                    k_sb[64:128, csl].bitcast(I32), k_sb[0:64, csl].bitcast(I32))

            def trp(c):
                for jj in range(4):
                    jc = 4 * c + jj
                    pt = mmp_pool.tile([128, H], BF16, tag="mm", name=f"pt{jc}")
                    nc.tensor.transpose(
                        pt[:], vt_sb[:, jc * 128:(jc + 1) * 128],
                        ident[0:64, 0:64])
                    nc.vector.tensor_copy(
                        vaug_sb[:, jc * (H + 1): jc * (H + 1) + H], pt[:])

            def q_proj(ck):
                # [wq|wq] stationary: one matmul per e-slice emits q into both
                # PSUM partition halves at once; a single [128, w] cast lands
                # q duplicated in both SBUF halves.
                w = QW[ck]
                base = QB[ck]
                pv = mmp_pool.tile([128, 512], F32, tag="mm", name=f"pv{ck}")
                for e in range(EC):
                    nc.tensor.matmul(
                        pv[:, 0:w], wq_sb[:, e * 128:(e + 1) * 128],
                        xq_all[:, base + e * w:base + (e + 1) * w],
                        start=(e == 0), stop=(e == EC - 1))
                lo = ck * 512 + (512 - w)
                nc.vector.tensor_copy(q_sb[:, lo:(ck + 1) * 512], pv[:, 0:w])

            # ---- attention machinery ----
            # mask tile packing offsets (shared layout; content is per-core)
            mask_offs = {}
            off = 0
            for ck in range(QKC):
                for jc in range(jmax[ck]):
                    qo, me = mm_off[ck][jc], mk_end[ck][jc]
                    if me > qo and qo < 512:
                        mask_offs[(ck, jc)] = off
                        off += me - qo

            tiles = {ck: [(jc, mm_off[ck][jc], mk_end[ck][jc])
                          for jc in range(jmax[ck]) if mm_off[ck][jc] < 512]
                     for ck in range(QKC)}
            po_state = {}  # ck -> [po_tile, n_av_done, n_av_total]
            pend = []      # delayed AV work: (ck, pair, p_t)

            def flush_av():
                # AVs run one pair-step late so the PE queue never waits on
                # the exp round-trip; the mask multiply (GpSimd) hides in the
                # same slack.
                while pend:
                    ck, pair, p_t = pend.pop(0)
                    st = po_state[ck]
                    for h, (jc, qo, me) in enumerate(pair):
                        nc.tensor.matmul(
                            st[0][:, qo:512],
                            vaug_sb[:, jc * (H + 1):(jc + 1) * (H + 1)],
                            p_t[:, h * 512 + qo:(h + 1) * 512],
                            start=(st[1] == 0), stop=(st[1] == st[2] - 1))
                        st[1] += 1

            def att_step(ck, pair):
                if ck not in po_state:
                    po_state[ck] = [po_pool.tile([H + 1, 512], F32, tag="po",
                                                 name=f"po{ck}"),
                                    0, len(tiles[ck])]
                ps = mms_pool.tile([128, 1024], F32, tag="mms", name="ps")
                p_t = p_pool.tile([128, 1024], BF16, tag="p", name="p_t")
                for h, (jc, qo, me) in enumerate(pair):
                    hf = 64 * (jc % 2)  # row-tiled: alternate array halves
                    nc.tensor.matmul(
                        ps[:, h * 512 + qo:(h + 1) * 512],
                        k_sb[hf:hf + 64, jc * 128:(jc + 1) * 128],
                        q_sb[hf:hf + 64, ck * 512 + qo:(ck + 1) * 512],
                        start=True, stop=True, skip_group_check=True)
                if len(pair) == 2 and pair[1][1] < 352:
                    lo = pair[0][1]
                    nc.scalar.activation(
                        p_t[:, lo:1024], ps[:, lo:1024],
                        mybir.ActivationFunctionType.Exp, scale=SCALE)
                else:
                    for h, (jc, qo, me) in enumerate(pair):
                        nc.scalar.activation(
                            p_t[:, h * 512 + qo:(h + 1) * 512],
                            ps[:, h * 512 + qo:(h + 1) * 512],
                            mybir.ActivationFunctionType.Exp, scale=SCALE)
                for h, (jc, qo, me) in enumerate(pair):
                    if me > qo:  # boundary mask multiply (host-built content)
                        mo = mask_offs[(ck, jc)]
                        nc.gpsimd.tensor_mul(
                            p_t[:, h * 512 + qo:h * 512 + me],
                            p_t[:, h * 512 + qo:h * 512 + me],
                            msk_sb[:, mo:mo + (me - qo)])
                return (ck, pair, p_t)

            def att(ck, j_lo, j_hi):
                seg = [t for t in tiles[ck] if j_lo <= t[0] <= j_hi]
                i = 0
                while i < len(seg):
                    pair = seg[i:i + 2]
                    item = att_step(ck, pair)
                    flush_av()
                    pend.append(item)
                    i += len(pair)

            def att_close(ck):
                # ship unnormalized outT + sums row; the host divides while
                # unsharding (removes the recip chain from the critical tail)
                flush_av()
                st = po_state[ck]
                assert st[1] == st[2], (ck, st[1], st[2])
                w0 = mm_off[ck][0]
                o_t = o_pool.tile([H + 1, 512], BF16, tag="o", name=f"o{ck}")
                if ck == 2:  # Scalar is idle once the last exp retired
                    nc.scalar.copy(o_t[:, w0:512], st[0][:, w0:512])
                else:
                    nc.vector.tensor_copy(o_t[:, w0:512], st[0][:, w0:512])
                nc.sync.dma_start(
                    out_ext.ap()[:, ck * 512 + w0:(ck + 1) * 512], o_t[:, w0:512])

            # ---- schedule: consumers emitted as their inputs land, AVs one
            # step late, kv/q/transpose work woven between attention steps ----
            kv_group(0)
            q_proj(0)
            q_proj(1)
            trp(0)
            att(0, 0, 0)
            att_close(0)
            att(1, 0, 3)
            q_proj(2)
            att(2, 0, 3)
            kv_group(1)
            trp(1)
            att(1, 4, 7)
            att(2, 4, 7)
            kv_group(2)
            trp(2)
            att(1, 8, 8)
            att_close(1)
            att(2, 8, 11)
            kv_group(3)
            trp(3)
            att(2, 12, 15)
            att_close(2)

    nc.compile()
    return nc


def _pack_masks(pos, jmax, mm_off, mk_end):
    """Per-core packed boundary masks: msk[j_local, off+q-qo] = (128jc + j_local <= pos[q])."""
    total = 0
    spans = []
    for ck in range(len(jmax)):
        for jc in range(jmax[ck]):
            qo, me = mm_off[ck][jc], mk_end[ck][jc]
            if me > qo and qo < 512:
                spans.append((ck, jc, qo, me, total))
                total += me - qo
    bf = ml_dtypes.bfloat16
    masks = np.zeros((B, 128, max(total, 1)), dtype=np.float32)
    jl = np.arange(128)[:, None]
    for b in range(B):
        for ck, jc, qo, me, off in spans:
            pq = pos[b, ck * 512 + qo: ck * 512 + me][None, :]
            masks[b, :, off:off + (me - qo)] = (128 * jc + jl <= pq)
    return masks.astype(bf), total


def _sbufify(w):  # [E, M] -> [128, EC*M]: w_t[p, e*M+m] = w[e*128+p, m]
    M = w.shape[1]
    return np.ascontiguousarray(
        w.reshape(EC, 128, M).transpose(1, 0, 2).reshape(128, EC * M))


def _retile_cols(xt, ncols, w=512):  # [E, ncols] -> [128, (ncols/w)*EC*w] chunk-major
    return np.ascontiguousarray(
        xt.reshape(EC, 128, ncols // w, w).transpose(1, 2, 0, 3)
        .reshape(128, (ncols // w) * EC * w))


def make_in_maps(x, Wq, Wk, Wv, zero_mask):
    x = np.asarray(x)
    pos, qoff, jmax, mm_off, mk_end = _plan(zero_mask)
    masks, mask_w = _pack_masks(pos, jmax, mm_off, mk_end)
    bf = ml_dtypes.bfloat16
    f8 = ml_dtypes.float8_e4m3fn
    W0F = mm_off[0][0] & ~3
    wkv = _sbufify(np.concatenate([np.asarray(Wk), np.asarray(Wv)], 1)).astype(bf)
    wq = _sbufify(np.concatenate([np.asarray(Wq), np.asarray(Wq)], 1)).astype(f8)
    maps = []
    for b in range(B):
        xtb = np.ascontiguousarray(x[b].T.astype(np.float32))
        xqb = np.zeros((E, QK_PAD), dtype=np.float32)
        real = pos[b] >= 0
        xqb[:, real] = xtb[:, pos[b][real]]
        # chunk 0 ships trimmed (cols W0F:512 only; the rest are pads on
        # every core), chunks 1-2 full width
        xq0 = np.ascontiguousarray(
            xqb[:, W0F:512].reshape(EC, 128, 512 - W0F)
            .transpose(1, 0, 2).reshape(128, EC * (512 - W0F))).astype(f8)
        xq12 = _retile_cols(xqb[:, 512:], 1024).astype(f8)
        xq_packed = np.concatenate(  # [wq | xq0 | masks | xq1 | xq2]
            [wq, xq0, masks[b].astype(f8), xq12], axis=1)
        xt_packed = np.concatenate([wkv, _retile_cols(xtb, C).astype(bf)], axis=1)
        maps.append({
            "xt": np.ascontiguousarray(xt_packed),
            "xq": np.ascontiguousarray(xq_packed),
        })
    return maps, (pos, jmax, mm_off, mk_end, mask_w)


def kernel(x, Wq, Wk, Wv, zero_mask):
    in_maps, (pos, jmax, mm_off, mk_end, mask_w) = make_in_maps(
        x, Wq, Wk, Wv, zero_mask)
    key = (jmax, tuple(map(tuple, mm_off)), tuple(map(tuple, mk_end)), mask_w)
    if _CACHED.get("key") != key:
        _CACHED["nc"] = _build(jmax, mm_off, mk_end, mask_w)
        _CACHED["key"] = key
    res = run_bass_kernel_spmd(_CACHED["nc"], in_maps, core_ids=list(range(B)))
    out = np.zeros((B, C, H), dtype=np.float32)
    for b in range(B):
        r = res.results[b]["out"].astype(np.float32)  # [H+1, QK_PAD]
        real = pos[b] >= 0
        out[b][pos[b][real]] = (r[:H, real] / r[H:H + 1, real]).T
    return out


# revision 14
# speedup vs baseline: 1.1044x; 1.0314x over previous
"""Causal attention head (B=8, C=2048, E=1024, H=64) with post-softmax query-row
zero mask, on 8 TRN2 NeuronCores — data-parallel over batch (one batch per core).

Sparse trick: ~50% of query rows are zero-masked post-softmax, so their outputs
are never needed. The host gathers the kept query positions (sorted), pads them
at the FRONT to a fixed QK_PAD, and the device computes attention only for
gathered query columns. Causality for gathered columns is enforced by
(a) restricting each score tile's moving range to columns whose position can
reach that key chunk (host-baked, min over cores) and (b) one narrow host-built
0/1 mask multiply per boundary tile (per-core data).

Pipeline layout (v2): DMA triggers are ordered so consumers unblock in
program order (xt0 halves, xq[wq|q0|masks|q1], xq[q2], xt1, xt2, xt3 halves);
attention j-segments are emitted between kv-projection chunks as their k/q
inputs land, so the PE queue never blocks on a far-future dependency and the
Scalar engine's exp stream (the attention-phase bottleneck, ~(N+352)/1.2 ns
per instruction) starts as early as possible and never drains. k is projected
once and duplicated into both SBUF partition halves by GpSimd (off the
PE/Vector critical path); q is projected into both halves at once by a
column-tiled self-pair matmul, so one [128,w] cast replaces two copies.

Per-core dataflow (matmuls bf16/fp8 -> f32 PSUM):
  [Wk|Wv] packed projection over all 2048 key positions -> k_sb, vt_sb.
  v^T PE-transposed into v_aug tiles [128j, 65] with column 64 = 1.0 (the AV
        matmul then also emits softmax denominators as row 64).
  Wq projection over gathered x columns (fp8; quantization error attenuated
        ~45x by the C**-0.5 score scale) -> q_sb.
  scoresT[j, q] = k-chunk (stationary, row-tiled across PE array halves) @ q
        (moving); exp on ScalarE with the C**-0.5 scale fused, two j-tiles per
        exp op when the garbage span is under 352 columns; AV accumulates
        outT[65, q] over j-chunks (row 64 = softmax denominators).
  The host divides by the denominators and scatters columns back to rows
  (masked rows stay zero) while unsharding. Output ships bf16 (unnormalized
  sums; the host divide restores precision headroom).
"""

import numpy as np
import ml_dtypes

import concourse.bass as bass
import concourse.bacc as bacc
import concourse.mybir as mybir
import concourse.tile as tile
from concourse.bass_utils import run_bass_kernel_spmd
from concourse.masks import make_identity

B, C, E, H = 8, 2048, 1024, 64
EC = E // 128          # 8 contraction chunks
KC = C // 512          # 4 key/value column chunks of 512
NJ = C // 128          # 16 key chunks of 128
QK_PAD = 1536          # gathered queries padded (front) to this
QKC = QK_PAD // 512    # 3 gathered-query chunks
SCALE = float(C) ** -0.5
BF16 = mybir.dt.bfloat16
F32 = mybir.dt.float32

_CACHED = {}


def _plan(zero_mask):
    """Host-side plan: per-core gathered positions + shared baked bounds."""
    zm = np.asarray(zero_mask)
    pos = []   # per core: [QK_PAD] int, -1 for front pads
    for b in range(B):
        kept = np.nonzero(~zm[b])[0]
        assert len(kept) <= QK_PAD, len(kept)
        p = np.full(QK_PAD, -1, dtype=np.int64)
        p[QK_PAD - len(kept):] = kept
        pos.append(p)
    pos = np.stack(pos)  # [B, QK_PAD]

    # qoff[b, ck, jc] = #cols in chunk ck with pos < 128*jc (cols are sorted)
    qoff = np.zeros((B, QKC, NJ + 1), dtype=np.int64)
    for ck in range(QKC):
        pc = pos[:, ck * 512:(ck + 1) * 512]
        for jc in range(NJ + 1):
            qoff[:, ck, jc] = (pc < 128 * jc).sum(axis=1)
    jmax = []   # per chunk: number of key chunks any core needs
    mm_off = []  # baked matmul start col (min over cores)
    mk_end = []  # baked mask end col (max over cores)
    for ck in range(QKC):
        jm = 0
        for jc in range(NJ):
            if (qoff[:, ck, jc] < 512).any():
                jm = jc + 1
        jmax.append(jm)
        mm_off.append([int(qoff[:, ck, jc].min()) for jc in range(NJ)])
        mk_end.append([int(qoff[:, ck, jc + 1].max()) for jc in range(NJ)])
    return pos, qoff, tuple(jmax), mm_off, mk_end


def _build(jmax, mm_off, mk_end, mask_w):
    nc = bacc.Bacc("TRN2", target_bir_lowering=False, debug=False, num_devices=B)
    FP8 = mybir.dt.float8e4
    MW = max(mask_w, 1)
    W0F = mm_off[0][0] & ~3       # pad columns trimmed from xq chunk 0
    QW = [512 - W0F, 512, 512]    # shipped query-chunk widths
    XOFF = EC * 128   # wkv columns at the head of xt
    QOFF = EC * 128   # [wq|wq] columns at the head of xq (duplicated: one
                      # matmul then emits q into both PSUM partition halves)
    # xq layout: [wq | xq0 (trimmed) | xq1 | masks | xq2]
    QB = [QOFF]                     # per-chunk column base inside xq
    QB.append(QOFF + EC * QW[0])
    MSK0 = QB[1] + EC * 512         # masks sit between xq1 and xq2
    QB.append(MSK0 + MW)
    XQW = QB[2] + EC * 512
    xt_ext = nc.dram_tensor("xt", [128, XOFF + KC * EC * 512], BF16, kind="ExternalInput")
    xq_ext = nc.dram_tensor("xq", [128, XQW], FP8, kind="ExternalInput")
    out_ext = nc.dram_tensor("out", [H + 1, QK_PAD], BF16, kind="ExternalOutput")

    with tile.TileContext(nc) as tc:
        with (
            tc.tile_pool(name="const", bufs=1) as const_pool,
            tc.tile_pool(name="acts", bufs=1) as act_pool,
            tc.tile_pool(name="p", bufs=4) as p_pool,
            tc.tile_pool(name="osb", bufs=2) as o_pool,
            tc.tile_pool(name="mmp", bufs=2, space="PSUM") as mmp_pool,
            tc.tile_pool(name="mms", bufs=2, space="PSUM") as mms_pool,
            tc.tile_pool(name="po", bufs=2, space="PSUM") as po_pool,
        ):
            ident = const_pool.tile([128, 128], BF16)
            xt_all = act_pool.tile([128, XOFF + KC * EC * 512], BF16)
            xq_all = act_pool.tile([128, XQW], FP8)
            msk_sb = xq_all[:, MSK0:MSK0 + MW]
            wkv_sb = xt_all[:, 0:XOFF]
            xt_sb = xt_all[:, XOFF:]
            wq_sb = xq_all[:, 0:QOFF]

            # ---- input DMA triggers, in consumer order (sync queue) ----
            half = EC * 512 // 2
            quart = EC * 512 // 4

            def xt_rng(lo, hi):
                nc.sync.dma_start(xt_all[:, lo:hi], xt_ext.ap()[:, lo:hi])

            def xq_rng(lo, hi):
                nc.sync.dma_start(xq_all[:, lo:hi], xq_ext.ap()[:, lo:hi])

            # xt0 lands in 4 paced sub-triggers: the PE starts on real work
            # ~3.5us in and its activity ramp keeps the HAM clock-gate from
            # re-throttling before the projection stream takes over
            xt_rng(0, XOFF + quart)                       # wkv + xt0 e0-1
            xt_rng(XOFF + quart, XOFF + half)             # xt0 e2-3
            xt_rng(XOFF + half, XOFF + 3 * quart)         # xt0 e4-5
            xt_rng(XOFF + 3 * quart, XOFF + EC * 512)     # xt0 e6-7
            xq_rng(0, QB[1])                              # wq + xq0
            xq_rng(QB[1], MSK0)                           # xq1
            xq_rng(MSK0, XQW)                             # masks + xq2
            xt_rng(XOFF + EC * 512, XOFF + 2 * EC * 512)  # xt1
            xt_rng(XOFF + 2 * EC * 512, XOFF + 3 * EC * 512)  # xt2
            xt_rng(XOFF + 3 * EC * 512, XOFF + 3 * EC * 512 + half)  # xt3 e0-3
            xt_rng(XOFF + 3 * EC * 512 + half, XOFF + 4 * EC * 512)  # xt3 e4-7

            make_identity(nc, ident[:])
            # touch Exp once so the ACT table set loads during the DMA phase
            warm = const_pool.tile([1, 1], F32)
            nc.scalar.activation(warm[:], ident[0:1, 0:1],
                                 mybir.ActivationFunctionType.Exp)

            # k and q live duplicated in both partition halves so score matmuls
            # (K=64) can run row-tiled: even j-chunks use array rows 0:64, odd
            # j-chunks rows 64:128 — the two halves compute concurrently.
            k_sb = act_pool.tile([128, C], BF16)
            vt_sb = act_pool.tile([64, C], BF16)
            q_sb = act_pool.tile([128, QK_PAD], BF16)
            vaug_sb = act_pool.tile([128, NJ * (H + 1)], BF16)
            nc.vector.memset(vaug_sb[:], 1.0)

            I32 = mybir.dt.int32

            def kv_group(c, vt_s=False, khi_s=False):
                csl = slice(c * 512, (c + 1) * 512)
                pq = mmp_pool.tile([128, 512], F32, tag="mm", name=f"pq{c}")
                for e in range(EC):
                    nc.tensor.matmul(
                        pq[:], wkv_sb[:, e * 128:(e + 1) * 128],
                        xt_sb[:, (c * EC + e) * 512:(c * EC + e + 1) * 512],
                        start=(e == 0), stop=(e == EC - 1))
                nc.vector.tensor_copy(k_sb[0:64, csl], pq[0:64, :])
                # vt/k-high land on Scalar exactly where its exp stream has a
                # k-availability gap; otherwise Vector / GpSimd carry them
                if vt_s:
                    nc.scalar.copy(vt_sb[:, csl], pq[64:128, :])
                else:
                    nc.vector.tensor_copy(vt_sb[:, csl], pq[64:128, :])
                if khi_s:
                    nc.scalar.copy(k_sb[64:128, csl], pq[0:64, :])
                else:
                    # SBUF->SBUF dup off the critical engines (int32 bitcast
                    # for the packed-copy fast path)
                    nc.gpsimd.tensor_copy(
                        k_sb[64:128, csl].bitcast(I32),
                        k_sb[0:64, csl].bitcast(I32))

            def trp(c):
                for jj in range(4):
                    jc = 4 * c + jj
                    pt = mmp_pool.tile([128, H], BF16, tag="mm", name=f"pt{jc}")
                    nc.tensor.transpose(
                        pt[:], vt_sb[:, jc * 128:(jc + 1) * 128],
                        ident[0:64, 0:64])
                    nc.vector.tensor_copy(
                        vaug_sb[:, jc * (H + 1): jc * (H + 1) + H], pt[:])

            def q_proj(ck):
                # [wq|wq] stationary: one matmul per e-slice emits q into both
                # PSUM partition halves at once; a single [128, w] cast lands
                # q duplicated in both SBUF halves.
                w = QW[ck]
                base = QB[ck]
                pv = mmp_pool.tile([128, 512], F32, tag="mm", name=f"pv{ck}")
                for e in range(EC):
                    nc.tensor.matmul(
                        pv[:, 0:w], wq_sb[:, e * 128:(e + 1) * 128],
                        xq_all[:, base + e * w:base + (e + 1) * w],
                        start=(e == 0), stop=(e == EC - 1))
                lo = ck * 512 + (512 - w)
                nc.vector.tensor_copy(q_sb[:, lo:(ck + 1) * 512], pv[:, 0:w])

            # ---- attention machinery ----
            # mask tile packing offsets (shared layout; content is per-core)
            mask_offs = {}
            off = 0
            for ck in range(QKC):
                for jc in range(jmax[ck]):
                    qo, me = mm_off[ck][jc], mk_end[ck][jc]
                    if me > qo and qo < 512:
                        mask_offs[(ck, jc)] = off
                        off += me - qo

            tiles = {ck: [(jc, mm_off[ck][jc], mk_end[ck][jc])
                          for jc in range(jmax[ck]) if mm_off[ck][jc] < 512]
                     for ck in range(QKC)}
            po_state = {}  # ck -> [po_tile, n_av_done, n_av_total]
            pend = []      # delayed AV work: (ck, pair, p_t)

            def flush_av():
                # AVs run one pair-step late so the PE queue never waits on
                # the exp round-trip; the mask multiply (GpSimd) hides in the
                # same slack.
                while pend:
                    ck, pair, p_t = pend.pop(0)
                    st = po_state[ck]
                    for h, (jc, qo, me) in enumerate(pair):
                        nc.tensor.matmul(
                            st[0][:, qo:512],
                            vaug_sb[:, jc * (H + 1):(jc + 1) * (H + 1)],
                            p_t[:, h * 512 + qo:(h + 1) * 512],
                            start=(st[1] == 0), stop=(st[1] == st[2] - 1))
                        st[1] += 1

            def att_step(ck, pair):
                if ck not in po_state:
                    po_state[ck] = [po_pool.tile([H + 1, 512], F32, tag="po",
                                                 name=f"po{ck}"),
                                    0, len(tiles[ck])]
                ps = mms_pool.tile([128, 1024], F32, tag="mms", name="ps")
                p_t = p_pool.tile([128, 1024], BF16, tag="p", name="p_t")
                for h, (jc, qo, me) in enumerate(pair):
                    hf = 64 * (jc % 2)  # row-tiled: alternate array halves
                    nc.tensor.matmul(
                        ps[:, h * 512 + qo:(h + 1) * 512],
                        k_sb[hf:hf + 64, jc * 128:(jc + 1) * 128],
                        q_sb[hf:hf + 64, ck * 512 + qo:(ck + 1) * 512],
                        start=True, stop=True, skip_group_check=True)
                if len(pair) == 2 and pair[1][1] < 352:
                    lo = pair[0][1]
                    nc.scalar.activation(
                        p_t[:, lo:1024], ps[:, lo:1024],
                        mybir.ActivationFunctionType.Exp, scale=SCALE)
                else:
                    for h, (jc, qo, me) in enumerate(pair):
                        nc.scalar.activation(
                            p_t[:, h * 512 + qo:(h + 1) * 512],
                            ps[:, h * 512 + qo:(h + 1) * 512],
                            mybir.ActivationFunctionType.Exp, scale=SCALE)
                for h, (jc, qo, me) in enumerate(pair):
                    if me > qo:  # boundary mask multiply (host-built content)
                        mo = mask_offs[(ck, jc)]
                        nc.gpsimd.tensor_mul(
                            p_t[:, h * 512 + qo:h * 512 + me],
                            p_t[:, h * 512 + qo:h * 512 + me],
                            msk_sb[:, mo:mo + (me - qo)])
                return (ck, pair, p_t)

            def att(ck, j_lo, j_hi):
                seg = [t for t in tiles[ck] if j_lo <= t[0] <= j_hi]
                i = 0
                while i < len(seg):
                    pair = seg[i:i + 2]
                    item = att_step(ck, pair)
                    flush_av()
                    pend.append(item)
                    i += len(pair)

            def att_close(ck):
                # ship unnormalized outT + sums row; the host divides while
                # unsharding (removes the recip chain from the critical tail)
                flush_av()
                st = po_state[ck]
                assert st[1] == st[2], (ck, st[1], st[2])
                w0 = mm_off[ck][0]
                o_t = o_pool.tile([H + 1, 512], BF16, tag="o", name=f"o{ck}")
                if ck == 2:  # Scalar is idle once the last exp retired, and
                    # as an HWDGE engine it fires the DMA with no sync handoff
                    nc.scalar.copy(o_t[:, w0:512], st[0][:, w0:512])
                    nc.scalar.dma_start(
                        out_ext.ap()[:, ck * 512 + w0:(ck + 1) * 512],
                        o_t[:, w0:512])
                else:
                    nc.vector.tensor_copy(o_t[:, w0:512], st[0][:, w0:512])
                    nc.sync.dma_start(
                        out_ext.ap()[:, ck * 512 + w0:(ck + 1) * 512],
                        o_t[:, w0:512])

            # ---- schedule: consumers emitted as their inputs land, AVs one
            # step late, kv/q/transpose work woven between attention steps ----
            kv_group(0, vt_s=True)
            q_proj(0)
            q_proj(1)
            trp(0)
            att(0, 0, 0)
            att_close(0)
            att(1, 0, 3)
            q_proj(2)
            att(2, 0, 3)
            kv_group(1, vt_s=True)
            trp(1)
            att(1, 4, 7)
            att(2, 4, 5)
            kv_group(2)
            att(2, 6, 7)
            trp(2)
            att(1, 8, 8)
            att_close(1)
            att(2, 8, 11)
            kv_group(3, khi_s=True)
            trp(3)
            att(2, 12, 15)
            att_close(2)

    nc.compile()
    return nc


def _pack_masks(pos, jmax, mm_off, mk_end):
    """Per-core packed boundary masks: msk[j_local, off+q-qo] = (128jc + j_local <= pos[q])."""
    total = 0
    spans = []
    for ck in range(len(jmax)):
        for jc in range(jmax[ck]):
            qo, me = mm_off[ck][jc], mk_end[ck][jc]
            if me > qo and qo < 512:
                spans.append((ck, jc, qo, me, total))
                total += me - qo
    bf = ml_dtypes.bfloat16
    masks = np.zeros((B, 128, max(total, 1)), dtype=np.float32)
    jl = np.arange(128)[:, None]
    for b in range(B):
        for ck, jc, qo, me, off in spans:
            pq = pos[b, ck * 512 + qo: ck * 512 + me][None, :]
            masks[b, :, off:off + (me - qo)] = (128 * jc + jl <= pq)
    return masks.astype(bf), total


def _sbufify(w):  # [E, M] -> [128, EC*M]: w_t[p, e*M+m] = w[e*128+p, m]
    M = w.shape[1]
    return np.ascontiguousarray(
        w.reshape(EC, 128, M).transpose(1, 0, 2).reshape(128, EC * M))


def _retile_cols(xt, ncols, w=512):  # [E, ncols] -> [128, (ncols/w)*EC*w] chunk-major
    return np.ascontiguousarray(
        xt.reshape(EC, 128, ncols // w, w).transpose(1, 2, 0, 3)
        .reshape(128, (ncols // w) * EC * w))


def make_in_maps(x, Wq, Wk, Wv, zero_mask):
    x = np.asarray(x)
    pos, qoff, jmax, mm_off, mk_end = _plan(zero_mask)
    masks, mask_w = _pack_masks(pos, jmax, mm_off, mk_end)
    bf = ml_dtypes.bfloat16
    f8 = ml_dtypes.float8_e4m3fn
    W0F = mm_off[0][0] & ~3
    wkv = _sbufify(np.concatenate([np.asarray(Wk), np.asarray(Wv)], 1)).astype(bf)
    wq = _sbufify(np.concatenate([np.asarray(Wq), np.asarray(Wq)], 1)).astype(f8)
    maps = []
    for b in range(B):
        xtb = np.ascontiguousarray(x[b].T.astype(np.float32))
        xqb = np.zeros((E, QK_PAD), dtype=np.float32)
        real = pos[b] >= 0
        xqb[:, real] = xtb[:, pos[b][real]]
        # chunk 0 ships trimmed (cols W0F:512 only; the rest are pads on
        # every core), chunks 1-2 full width
        xq0 = np.ascontiguousarray(
            xqb[:, W0F:512].reshape(EC, 128, 512 - W0F)
            .transpose(1, 0, 2).reshape(128, EC * (512 - W0F))).astype(f8)
        xq1 = _retile_cols(xqb[:, 512:1024], 512).astype(f8)
        xq2 = _retile_cols(xqb[:, 1024:1536], 512).astype(f8)
        xq_packed = np.concatenate(  # [wq | xq0 | xq1 | masks | xq2]
            [wq, xq0, xq1, masks[b].astype(f8), xq2], axis=1)
        xt_packed = np.concatenate([wkv, _retile_cols(xtb, C).astype(bf)], axis=1)
        maps.append({
            "xt": np.ascontiguousarray(xt_packed),
            "xq": np.ascontiguousarray(xq_packed),
        })
    return maps, (pos, jmax, mm_off, mk_end, mask_w)


def kernel(x, Wq, Wk, Wv, zero_mask):
    in_maps, (pos, jmax, mm_off, mk_end, mask_w) = make_in_maps(
        x, Wq, Wk, Wv, zero_mask)
    key = (jmax, tuple(map(tuple, mm_off)), tuple(map(tuple, mk_end)), mask_w)
    if _CACHED.get("key") != key:
        _CACHED["nc"] = _build(jmax, mm_off, mk_end, mask_w)
        _CACHED["key"] = key
    res = run_bass_kernel_spmd(_CACHED["nc"], in_maps, core_ids=list(range(B)))
    out = np.zeros((B, C, H), dtype=np.float32)
    for b in range(B):
        r = res.results[b]["out"].astype(np.float32)  # [H+1, QK_PAD]
        real = pos[b] >= 0
        out[b][pos[b][real]] = (r[:H, real] / r[H:H + 1, real]).T
    return out


# revision 20
# speedup vs baseline: 1.3852x; 1.2543x over previous
"""Causal attention head (B=8, C=2048, E=1024, H=64) with post-softmax query-row
zero mask, on 8 TRN2 NeuronCores — data-parallel over batch (one batch per core).

Sparse trick: ~50% of query rows are zero-masked post-softmax, so their outputs
are never needed. The host gathers the kept query positions (sorted), pads them
at the FRONT to a fixed QK_PAD, and the device computes attention only for
gathered query columns. Causality for gathered columns is enforced by
(a) restricting each score tile's moving range to columns whose position can
reach that key chunk (host-baked, min over cores) and (b) one narrow host-built
0/1 mask multiply per boundary tile (per-core data).

Pipeline layout (v2): DMA triggers are ordered so consumers unblock in
program order (xt0 halves, xq[wq|q0|masks|q1], xq[q2], xt1, xt2, xt3 halves);
attention j-segments are emitted between kv-projection chunks as their k/q
inputs land, so the PE queue never blocks on a far-future dependency and the
Scalar engine's exp stream (the attention-phase bottleneck, ~(N+352)/1.2 ns
per instruction) starts as early as possible and never drains. k is projected
once and duplicated into both SBUF partition halves by GpSimd (off the
PE/Vector critical path); q is projected into both halves at once by a
column-tiled self-pair matmul, so one [128,w] cast replaces two copies.

Per-core dataflow (matmuls bf16/fp8 -> f32 PSUM):
  [Wk|Wv] packed projection over all 2048 key positions -> k_sb, vt_sb.
  v^T PE-transposed into v_aug tiles [128j, 65] with column 64 = 1.0 (the AV
        matmul then also emits softmax denominators as row 64).
  Wq projection over gathered x columns (fp8; quantization error attenuated
        ~45x by the C**-0.5 score scale) -> q_sb.
  scoresT[j, q] = k-chunk (stationary, row-tiled across PE array halves) @ q
        (moving); exp on ScalarE with the C**-0.5 scale fused, two j-tiles per
        exp op when the garbage span is under 352 columns; AV accumulates
        outT[65, q] over j-chunks (row 64 = softmax denominators).
  The host divides by the denominators and scatters columns back to rows
  (masked rows stay zero) while unsharding. Output ships bf16 (unnormalized
  sums; the host divide restores precision headroom).
"""

import numpy as np
import ml_dtypes

import concourse.bass as bass
import concourse.bacc as bacc
import concourse.mybir as mybir
import concourse.tile as tile
from concourse.bass_utils import run_bass_kernel_spmd
from concourse.masks import make_identity

B, C, E, H = 8, 2048, 1024, 64
EC = E // 128          # 8 contraction chunks
KC = C // 512          # 4 key/value column chunks of 512
NJ = C // 128          # 16 key chunks of 128
QK_PAD = 1536          # gathered queries padded (front) to this
QKC = QK_PAD // 512    # 3 gathered-query chunks
SCALE = float(C) ** -0.5
BF16 = mybir.dt.bfloat16
F32 = mybir.dt.float32

_CACHED = {}


def _plan(zero_mask):
    """Host-side plan: per-core gathered positions + shared baked bounds."""
    zm = np.asarray(zero_mask)
    pos = []   # per core: [QK_PAD] int, -1 for front pads
    for b in range(B):
        kept = np.nonzero(~zm[b])[0]
        assert len(kept) <= QK_PAD, len(kept)
        p = np.full(QK_PAD, -1, dtype=np.int64)
        p[QK_PAD - len(kept):] = kept
        pos.append(p)
    pos = np.stack(pos)  # [B, QK_PAD]

    # qoff[b, ck, jc] = #cols in chunk ck with pos < 128*jc (cols are sorted)
    qoff = np.zeros((B, QKC, NJ + 1), dtype=np.int64)
    for ck in range(QKC):
        pc = pos[:, ck * 512:(ck + 1) * 512]
        for jc in range(NJ + 1):
            qoff[:, ck, jc] = (pc < 128 * jc).sum(axis=1)
    jmax = []   # per chunk: number of key chunks any core needs
    mm_off = []  # baked matmul start col (min over cores)
    mk_end = []  # baked mask end col (max over cores)
    for ck in range(QKC):
        jm = 0
        for jc in range(NJ):
            if (qoff[:, ck, jc] < 512).any():
                jm = jc + 1
        jmax.append(jm)
        mm_off.append([int(qoff[:, ck, jc].min()) for jc in range(NJ)])
        mk_end.append([int(qoff[:, ck, jc + 1].max()) for jc in range(NJ)])
    return pos, qoff, tuple(jmax), mm_off, mk_end


def _build(jmax, mm_off, mk_end, mask_w):
    nc = bacc.Bacc("TRN2", target_bir_lowering=False, debug=False, num_devices=B)
    FP8 = mybir.dt.float8e4
    MW = max(mask_w, 1)
    W0F = mm_off[0][0] & ~3       # pad columns trimmed from xq chunk 0
    QW = [512 - W0F, 512, 512]    # shipped query-chunk widths
    XOFF = EC * 128   # wkv columns at the head of xt
    QOFF = EC * 128   # [wq|wq] columns at the head of xq (duplicated: one
                      # matmul then emits q into both PSUM partition halves)
    # xq layout: [wq | xq0 (trimmed) | xq1 | masks | xq2]
    QB = [QOFF]                     # per-chunk column base inside xq
    QB.append(QOFF + EC * QW[0])
    MSK0 = QB[1] + EC * 512         # masks sit between xq1 and xq2
    QB.append(MSK0 + MW)
    XQW = QB[2] + EC * 512
    xt_ext = nc.dram_tensor("xt", [128, XOFF + KC * EC * 512], BF16, kind="ExternalInput")
    xq_ext = nc.dram_tensor("xq", [128, XQW], FP8, kind="ExternalInput")
    out_ext = nc.dram_tensor("out", [H + 1, QK_PAD], BF16, kind="ExternalOutput")

    with tile.TileContext(nc) as tc:
        with (
            tc.tile_pool(name="const", bufs=1) as const_pool,
            tc.tile_pool(name="acts", bufs=1) as act_pool,
            tc.tile_pool(name="p", bufs=4) as p_pool,
            tc.tile_pool(name="osb", bufs=2) as o_pool,
            tc.tile_pool(name="mmp", bufs=2, space="PSUM") as mmp_pool,
            tc.tile_pool(name="mms", bufs=2, space="PSUM") as mms_pool,
            tc.tile_pool(name="po", bufs=2, space="PSUM") as po_pool,
        ):
            ident = const_pool.tile([128, 128], BF16)
            xt_all = act_pool.tile([128, XOFF + KC * EC * 512], BF16)
            xq_all = act_pool.tile([128, XQW], FP8)
            msk_sb = xq_all[:, MSK0:MSK0 + MW]
            wkv_sb = xt_all[:, 0:XOFF]
            xt_sb = xt_all[:, XOFF:]
            wq_sb = xq_all[:, 0:QOFF]

            # ---- input DMA triggers, in consumer order (sync queue) ----
            half = EC * 512 // 2
            quart = EC * 512 // 4

            def xt_rng(lo, hi):
                nc.sync.dma_start(xt_all[:, lo:hi], xt_ext.ap()[:, lo:hi])

            def xq_rng(lo, hi):
                nc.sync.dma_start(xq_all[:, lo:hi], xq_ext.ap()[:, lo:hi])

            xt_rng(0, XOFF + half)                        # wkv + xt0 e0-3
            xt_rng(XOFF + half, XOFF + EC * 512)          # xt0 e4-7
            xq_rng(0, QB[1])                              # wq + xq0
            xq_rng(QB[1], MSK0)                           # xq1
            xq_rng(MSK0, XQW)                             # masks + xq2
            xt_rng(XOFF + EC * 512, XOFF + 2 * EC * 512)  # xt1
            xt_rng(XOFF + 2 * EC * 512, XOFF + 3 * EC * 512)  # xt2
            xt_rng(XOFF + 3 * EC * 512, XOFF + 3 * EC * 512 + half)  # xt3 e0-3
            xt_rng(XOFF + 3 * EC * 512 + half, XOFF + 4 * EC * 512)  # xt3 e4-7

            make_identity(nc, ident[:])
            # touch Exp once so the ACT table set loads during the DMA phase
            warm = const_pool.tile([1, 1], F32)
            nc.scalar.activation(warm[:], ident[0:1, 0:1],
                                 mybir.ActivationFunctionType.Exp)

            # k and q live in partitions 0:64 only; scores always run on PE
            # array rows 0:64 (serial, but keeps every k-chunk handoff a
            # single Vector cast — no cross-engine duplication chain)
            k_sb = act_pool.tile([64, C], BF16)
            vt_sb = act_pool.tile([64, C], BF16)
            q_sb = act_pool.tile([64, QK_PAD], BF16)
            vaug_sb = act_pool.tile([128, NJ * (H + 1)], BF16)
            nc.vector.memset(vaug_sb[:], 1.0)

            I32 = mybir.dt.int32

            def kv_group(c):
                csl = slice(c * 512, (c + 1) * 512)
                pq = mmp_pool.tile([128, 512], F32, tag="mm", name=f"pq{c}")
                for e in range(EC):
                    nc.tensor.matmul(
                        pq[:], wkv_sb[:, e * 128:(e + 1) * 128],
                        xt_sb[:, (c * EC + e) * 512:(c * EC + e + 1) * 512],
                        start=(e == 0), stop=(e == EC - 1))
                nc.vector.tensor_copy(k_sb[:, csl], pq[0:64, :])
                nc.vector.tensor_copy(vt_sb[:, csl], pq[64:128, :])

            def trp(c):
                for jj in range(4):
                    jc = 4 * c + jj
                    pt = mmp_pool.tile([128, H], BF16, tag="mm", name=f"pt{jc}")
                    nc.tensor.transpose(
                        pt[:], vt_sb[:, jc * 128:(jc + 1) * 128],
                        ident[0:64, 0:64])
                    nc.vector.tensor_copy(
                        vaug_sb[:, jc * (H + 1): jc * (H + 1) + H], pt[:])

            def q_proj(ck):
                # [wq|wq] stationary: one matmul per e-slice emits q into both
                # PSUM partition halves at once; a single [128, w] cast lands
                # q duplicated in both SBUF halves.
                w = QW[ck]
                base = QB[ck]
                pv = mmp_pool.tile([128, 512], F32, tag="mm", name=f"pv{ck}")
                for e in range(EC):
                    nc.tensor.matmul(
                        pv[:, 0:w], wq_sb[:, e * 128:(e + 1) * 128],
                        xq_all[:, base + e * w:base + (e + 1) * w],
                        start=(e == 0), stop=(e == EC - 1))
                lo = ck * 512 + (512 - w)
                nc.vector.tensor_copy(q_sb[:, lo:(ck + 1) * 512], pv[0:64, 0:w])

            # ---- attention machinery ----
            # mask tile packing offsets (shared layout; content is per-core)
            mask_offs = {}
            off = 0
            for ck in range(QKC):
                for jc in range(jmax[ck]):
                    qo, me = mm_off[ck][jc], mk_end[ck][jc]
                    if me > qo and qo < 512:
                        mask_offs[(ck, jc)] = off
                        off += me - qo

            tiles = {ck: [(jc, mm_off[ck][jc], mk_end[ck][jc])
                          for jc in range(jmax[ck]) if mm_off[ck][jc] < 512]
                     for ck in range(QKC)}
            po_state = {}  # ck -> [po_tile, n_av_done, n_av_total]
            pend = []      # delayed AV work: (ck, pair, p_t)

            def flush_av():
                # AVs run one pair-step late so the PE queue never waits on
                # the exp round-trip; the mask multiply (GpSimd) hides in the
                # same slack.
                while pend:
                    ck, pair, p_t = pend.pop(0)
                    st = po_state[ck]
                    for h, (jc, qo, me) in enumerate(pair):
                        nc.tensor.matmul(
                            st[0][:, qo:512],
                            vaug_sb[:, jc * (H + 1):(jc + 1) * (H + 1)],
                            p_t[:, h * 512 + qo:(h + 1) * 512],
                            start=(st[1] == 0), stop=(st[1] == st[2] - 1))
                        st[1] += 1

            def att_step(ck, pair):
                if ck not in po_state:
                    po_state[ck] = [po_pool.tile([H + 1, 512], F32, tag="po",
                                                 name=f"po{ck}"),
                                    0, len(tiles[ck])]
                ps = mms_pool.tile([128, 1024], F32, tag="mms", name="ps")
                p_t = p_pool.tile([128, 1024], BF16, tag="p", name="p_t")
                for h, (jc, qo, me) in enumerate(pair):
                    nc.tensor.matmul(
                        ps[:, h * 512 + qo:(h + 1) * 512],
                        k_sb[:, jc * 128:(jc + 1) * 128],
                        q_sb[:, ck * 512 + qo:(ck + 1) * 512],
                        start=True, stop=True, skip_group_check=True)
                if len(pair) == 2 and pair[1][1] < 352:
                    lo = pair[0][1]
                    nc.scalar.activation(
                        p_t[:, lo:1024], ps[:, lo:1024],
                        mybir.ActivationFunctionType.Exp, scale=SCALE)
                else:
                    for h, (jc, qo, me) in enumerate(pair):
                        nc.scalar.activation(
                            p_t[:, h * 512 + qo:(h + 1) * 512],
                            ps[:, h * 512 + qo:(h + 1) * 512],
                            mybir.ActivationFunctionType.Exp, scale=SCALE)
                for h, (jc, qo, me) in enumerate(pair):
                    if me > qo:  # boundary mask multiply (host-built content)
                        mo = mask_offs[(ck, jc)]
                        nc.gpsimd.tensor_mul(
                            p_t[:, h * 512 + qo:h * 512 + me],
                            p_t[:, h * 512 + qo:h * 512 + me],
                            msk_sb[:, mo:mo + (me - qo)])
                return (ck, pair, p_t)

            def att(ck, j_lo, j_hi):
                seg = [t for t in tiles[ck] if j_lo <= t[0] <= j_hi]
                i = 0
                while i < len(seg):
                    pair = seg[i:i + 2]
                    item = att_step(ck, pair)
                    flush_av()
                    pend.append(item)
                    i += len(pair)

            def att_close(ck):
                # ship unnormalized outT + sums row; the host divides while
                # unsharding (removes the recip chain from the critical tail)
                flush_av()
                st = po_state[ck]
                assert st[1] == st[2], (ck, st[1], st[2])
                w0 = mm_off[ck][0]
                o_t = o_pool.tile([H + 1, 512], BF16, tag="o", name=f"o{ck}")
                if ck == 2:  # Scalar is idle once the last exp retired, and
                    # as an HWDGE engine it fires the DMA with no sync handoff
                    nc.scalar.copy(o_t[:, w0:512], st[0][:, w0:512])
                    nc.scalar.dma_start(
                        out_ext.ap()[:, ck * 512 + w0:(ck + 1) * 512],
                        o_t[:, w0:512])
                else:
                    nc.vector.tensor_copy(o_t[:, w0:512], st[0][:, w0:512])
                    nc.sync.dma_start(
                        out_ext.ap()[:, ck * 512 + w0:(ck + 1) * 512],
                        o_t[:, w0:512])

            # ---- schedule: consumers emitted as their inputs land, AVs one
            # step late, kv/q/transpose work woven between attention steps ----
            kv_group(0)
            q_proj(1)
            q_proj(2)
            q_proj(0)
            trp(0)
            att(0, 0, 0)
            att_close(0)
            att(1, 0, 3)
            att(2, 0, 3)
            kv_group(1)
            trp(1)
            att(1, 4, 7)
            att(2, 4, 7)
            kv_group(2)
            trp(2)
            att(1, 8, 8)
            att_close(1)
            att(2, 8, 11)
            kv_group(3)
            trp(3)
            att(2, 12, 15)
            att_close(2)

    nc.compile()
    return nc


def _pack_masks(pos, jmax, mm_off, mk_end):
    """Per-core packed boundary masks: msk[j_local, off+q-qo] = (128jc + j_local <= pos[q])."""
    total = 0
    spans = []
    for ck in range(len(jmax)):
        for jc in range(jmax[ck]):
            qo, me = mm_off[ck][jc], mk_end[ck][jc]
            if me > qo and qo < 512:
                spans.append((ck, jc, qo, me, total))
                total += me - qo
    bf = ml_dtypes.bfloat16
    masks = np.zeros((B, 128, max(total, 1)), dtype=np.float32)
    jl = np.arange(128)[:, None]
    for b in range(B):
        for ck, jc, qo, me, off in spans:
            pq = pos[b, ck * 512 + qo: ck * 512 + me][None, :]
            masks[b, :, off:off + (me - qo)] = (128 * jc + jl <= pq)
    return masks.astype(bf), total


def _sbufify(w):  # [E, M] -> [128, EC*M]: w_t[p, e*M+m] = w[e*128+p, m]
    M = w.shape[1]
    return np.ascontiguousarray(
        w.reshape(EC, 128, M).transpose(1, 0, 2).reshape(128, EC * M))


def _retile_cols(xt, ncols, w=512):  # [E, ncols] -> [128, (ncols/w)*EC*w] chunk-major
    return np.ascontiguousarray(
        xt.reshape(EC, 128, ncols // w, w).transpose(1, 2, 0, 3)
        .reshape(128, (ncols // w) * EC * w))


def make_in_maps(x, Wq, Wk, Wv, zero_mask):
    x = np.asarray(x)
    pos, qoff, jmax, mm_off, mk_end = _plan(zero_mask)
    masks, mask_w = _pack_masks(pos, jmax, mm_off, mk_end)
    bf = ml_dtypes.bfloat16
    f8 = ml_dtypes.float8_e4m3fn
    W0F = mm_off[0][0] & ~3
    wkv = _sbufify(np.concatenate([np.asarray(Wk), np.asarray(Wv)], 1)).astype(bf)
    wq = _sbufify(np.concatenate([np.asarray(Wq), np.asarray(Wq)], 1)).astype(f8)
    maps = []
    for b in range(B):
        xtb = np.ascontiguousarray(x[b].T.astype(np.float32))
        xqb = np.zeros((E, QK_PAD), dtype=np.float32)
        real = pos[b] >= 0
        xqb[:, real] = xtb[:, pos[b][real]]
        # chunk 0 ships trimmed (cols W0F:512 only; the rest are pads on
        # every core), chunks 1-2 full width
        xq0 = np.ascontiguousarray(
            xqb[:, W0F:512].reshape(EC, 128, 512 - W0F)
            .transpose(1, 0, 2).reshape(128, EC * (512 - W0F))).astype(f8)
        xq1 = _retile_cols(xqb[:, 512:1024], 512).astype(f8)
        xq2 = _retile_cols(xqb[:, 1024:1536], 512).astype(f8)
        xq_packed = np.concatenate(  # [wq | xq0 | xq1 | masks | xq2]
            [wq, xq0, xq1, masks[b].astype(f8), xq2], axis=1)
        xt_packed = np.concatenate([wkv, _retile_cols(xtb, C).astype(bf)], axis=1)
        maps.append({
            "xt": np.ascontiguousarray(xt_packed),
            "xq": np.ascontiguousarray(xq_packed),
        })
    return maps, (pos, jmax, mm_off, mk_end, mask_w)


def kernel(x, Wq, Wk, Wv, zero_mask):
    in_maps, (pos, jmax, mm_off, mk_end, mask_w) = make_in_maps(
        x, Wq, Wk, Wv, zero_mask)
    key = (jmax, tuple(map(tuple, mm_off)), tuple(map(tuple, mk_end)), mask_w)
    if _CACHED.get("key") != key:
        _CACHED["nc"] = _build(jmax, mm_off, mk_end, mask_w)
        _CACHED["key"] = key
    res = run_bass_kernel_spmd(_CACHED["nc"], in_maps, core_ids=list(range(B)))
    out = np.zeros((B, C, H), dtype=np.float32)
    for b in range(B):
        r = res.results[b]["out"].astype(np.float32)  # [H+1, QK_PAD]
        real = pos[b] >= 0
        out[b][pos[b][real]] = (r[:H, real] / r[H:H + 1, real]).T
    return out
